# revision 1
# baseline (speedup 1.0000x reference)
"""Multi-head attention (B=1, T=1500, N=1280, H=20, D=64) on 8 NeuronCores.

Tensor-parallel by head groups, balanced 2.5 heads/core: each core owns two
full heads (2c, 2c+1) plus half of a shared head (16 + c//2; q-rows
750*(c%2) .. +750).  Per core:
  - q/k projections computed transposed (qT = Wq_slice @ x.T) so scores
    matmuls contract over d with no on-chip transposes; the shared head's
    half-q is precomputed on the host (tiny) and passed in as `qs`,
  - v projection computed in natural [t, d] layout with an extra ones column
    so the attention wv matmul also yields the softmax denominator row,
  - exp without max subtraction (scores are ~N(0,1); max < ~7, safe in fp32),
  - float32r matmuls throughout (fp32 bits, full-rate PE path),
  - full-head out-projection partial -> `out` [1500,1280]; shared-head
    contribution -> `out2` [768,1280] (rows 750:768 are padding garbage),
  - the 8 partials are combined on device: psum(out + place(out2[:750])).
"""

import os

import numpy as np

T, F, D = 1500, 1280, 64
NH = 20
HS = 3              # head slots per core (2 full + 1 shared-half)
QK_SCALE = D ** (-0.5)
TC = 500            # projection chunk width
NTC = 3
PT = [128] * 11 + [92]      # partition tiles along t / kj (sum = 1500)
PT_OFF = [128 * i for i in range(12)]
ECH = [512, 384, 384]       # out-proj free-dim chunks (sum = 1280)
NCORES = 8
TQ = 768            # shared-head padded q width (750 real + 18 pad)

_CACHE = {}
LAST_RESULTS = None


def _build(mm_dtype_name):
    import concourse.bacc as bacc
    import concourse.mybir as mybir
    import concourse.tile as tile

    f32 = mybir.dt.float32
    mm_dt = getattr(mybir.dt, mm_dtype_name)

    nc = bacc.Bacc("TRN2", target_bir_lowering=False, debug=False,
                   num_devices=NCORES)

    xT_d = nc.dram_tensor("xT", [F, T], mm_dt, kind="ExternalInput").ap()
    wqk_d = nc.dram_tensor("wqk", [F, 384], mm_dt, kind="ExternalInput").ap()
    bqk_d = nc.dram_tensor("bqk", [384, 1], f32, kind="ExternalInput").ap()
    wvw_d = nc.dram_tensor("wvw", [F, 256], mm_dt, kind="ExternalInput").ap()
    bv_d = nc.dram_tensor("bv", [128, 256], f32, kind="ExternalInput").ap()
    qs_d = nc.dram_tensor("qs", [64, 768], mm_dt, kind="ExternalInput").ap()
    wo_d = nc.dram_tensor("wo", [HS * D, F], mm_dt, kind="ExternalInput").ap()
    out_d = nc.dram_tensor("out", [T, F], f32, kind="ExternalOutput").ap()
    out2_d = nc.dram_tensor("out2", [768, F], f32, kind="ExternalOutput").ap()

    EXP = mybir.ActivationFunctionType.Exp

    with tile.TileContext(nc) as tc:
        from contextlib import ExitStack
        with ExitStack() as ctx:
            persist = ctx.enter_context(tc.tile_pool(name="persist", bufs=1))

            # persistent SBUF tiles
            Q12 = persist.tile([128, T], mm_dt, tag="q12", name="q12")
            K12 = persist.tile([128, T], mm_dt, tag="k12", name="k12")
            K3 = persist.tile([128, T], mm_dt, tag="k3", name="k3")
            QS = persist.tile([64, 768], mm_dt, tag="qst", name="qst")
            # V: 12 t-tiles x 256 cols; per t-tile: [v|1] x 3 slots + pad
            V = persist.tile([128, 12 * 256], mm_dt, tag="vall", name="vall")
            WVT12 = persist.tile([128, T], mm_dt, tag="wvt12", name="wvt12")
            WVT3 = persist.tile([64, 768], mm_dt, tag="wvt3", name="wvt3")
            WO12 = persist.tile([128, F], mm_dt, tag="wo12", name="wo12")
            WO3 = persist.tile([64, F], mm_dt, tag="wo3", name="wo3")
            BV = persist.tile([128, 256], f32, tag="bv", name="bv")
            BQK = [persist.tile([128, 1], f32, tag=f"bqk{i}", name=f"bqk{i}")
                   for i in range(3)]

            # ---------------- projections ----------------
            with tc.tile_pool(name="xw", bufs=1) as xw, \
                 tc.tile_pool(name="pp_qk", bufs=4, space="PSUM") as pp_qk, \
                 tc.tile_pool(name="pp_v", bufs=3, space="PSUM") as pp_v:
                XT = [xw.tile([128, T], mm_dt, tag=f"xt{i}", name=f"xt{i}")
                      for i in range(10)]
                WQKW = [xw.tile([128, 384], mm_dt, tag=f"wqkw{i}",
                                name=f"wqkw{i}") for i in range(10)]
                WVW = [xw.tile([128, 256], mm_dt, tag=f"wvw{i}",
                               name=f"wvw{i}") for i in range(10)]
                # interleave weight/x tiles in consumption order so the
                # first accumulation group streams right behind the DMAs
                PCH = [(0, 512), (512, 512), (1024, 476)]
                for i in range(10):
                    nc.sync.dma_start(WQKW[i][:],
                                      wqk_d[128 * i:128 * (i + 1), :])
                    nc.sync.dma_start(
                        XT[i][:, 0:512], xT_d[128 * i:128 * (i + 1), 0:512])
                for m in range(3):
                    nc.sync.dma_start(BQK[m][:],
                                      bqk_d[128 * m:128 * (m + 1), :])
                nc.sync.dma_start(QS[:], qs_d[:, :])
                for i in range(10):
                    nc.sync.dma_start(WVW[i][:],
                                      wvw_d[128 * i:128 * (i + 1), :])
                nc.sync.dma_start(BV[:], bv_d[:, :])
                for (o, w) in PCH[1:]:
                    for i in range(10):
                        nc.sync.dma_start(
                            XT[i][:, o:o + w],
                            xT_d[128 * i:128 * (i + 1), o:o + w])
                nc.sync.dma_start(WO12[:], wo_d[0:128, :])
                nc.sync.dma_start(WO3[:], wo_d[128:192, :])

                # transposed q/k projections: m0=[q F0|q F1], m1=[k F0|k F1],
                # m2=[k S|pad]
                QKDEST = [Q12, K12, K3]
                for ci, (o, w) in enumerate(PCH):
                    for m in (1, 0, 2):
                        ps = pp_qk.tile([128, 512], f32, tag="pqk",
                                        name="pqk")
                        for f in range(10):
                            nc.tensor.matmul(
                                ps[0:128, 0:w],
                                WQKW[f][:, 128 * m:128 * (m + 1)],
                                XT[f][:, o:o + w],
                                start=(f == 0), stop=(f == 9),
                            )
                        nc.vector.tensor_scalar_add(
                            QKDEST[m][:, o:o + w], ps[0:128, 0:w],
                            BQK[m][:])
                    # v-proj t-tiles covered by this chunk (512-aligned)
                    for tt in range(4 * ci, min(4 * ci + 4, 12)):
                        pk = PT[tt]
                        ps = pp_v.tile([128, 256], f32, tag="pv", name="pv")
                        for f in range(10):
                            nc.tensor.matmul(
                                ps[0:pk, :],
                                XT[f][:, PT_OFF[tt]:PT_OFF[tt] + pk],
                                WVW[f][:],
                                start=(f == 0), stop=(f == 9),
                            )
                        nc.vector.tensor_add(
                            V[0:pk, 256 * tt:256 * (tt + 1)], ps[0:pk, :],
                            BV[0:pk, :])

            # ---------------- attention ----------------
            import concourse.bass as bass

            FULLSUB = [(0, 512), (512, 512), (1024, 476)]
            HALFSUB = [(0, 512), (512, 256)]

            from contextlib import ExitStack as _ES
            attn_ctx = _ES()
            epool = attn_ctx.enter_context(tc.tile_pool(name="epool", bufs=3))
            fin = attn_ctx.enter_context(tc.tile_pool(name="fin", bufs=6))
            zdram = attn_ctx.enter_context(
                tc.tile_pool(name="zdram", bufs=6, space="DRAM"))
            pp_stA = attn_ctx.enter_context(
                tc.tile_pool(name="pp_stA", bufs=2, space="PSUM"))
            pp_wva = attn_ctx.enter_context(
                tc.tile_pool(name="pp_wva", bufs=1, space="PSUM"))
            # stB + the third wv accumulator are only used by the two full
            # slots; scoping them lets the out-proj psum pool reuse their
            # banks while slot 2 still runs
            s01_ctx = _ES()
            pp_stB = s01_ctx.enter_context(
                tc.tile_pool(name="pp_stB", bufs=1, space="PSUM"))
            pp_wvb = s01_ctx.enter_context(
                tc.tile_pool(name="pp_wvb", bufs=1, space="PSUM"))

            if True:
                for s in range(HS):
                    if s < 2:
                        qt, kt, row0, sub = Q12, K12, 64 * s, FULLSUB
                        qrow = row0
                    else:
                        s01_ctx.close()   # free stB + wv2 banks for pp_o
                        qt, kt, row0, sub = QS, K3, 0, HALFSUB
                        qrow = 0
                    if s < 2:
                        wv_ps = [
                            pp_wva.tile([66, 512], f32, tag="pwv0",
                                        name="pwv0"),
                            pp_wva.tile([66, 512], f32, tag="pwv1",
                                        name="pwv1"),
                            pp_wvb.tile([66, 476], f32, tag="pwv2",
                                        name="pwv2"),
                        ]
                    else:
                        # slot 2 (after stB/wvb closed) uses only two chunks
                        wv_ps = [
                            pp_wva.tile([66, 512], f32, tag="pwv0",
                                        name="pwv0"),
                            pp_wva.tile([66, 512], f32, tag="pwv1",
                                        name="pwv1"),
                        ]
                    for tt in range(12):
                        pk = PT[tt]
                        E = epool.tile([128, T], mm_dt, tag="E", name="E")
                        psA = pp_stA.tile([128, 1024], f32, tag="pstA",
                                          name="pstA")
                        psB = (pp_stB.tile([128, 476], f32, tag="pstB",
                                           name="pstB") if s < 2 else None)
                        for (o, w) in sub:
                            dst = (psA[0:pk, o:o + w] if o < 1024
                                   else psB[0:pk, 0:w])
                            nc.tensor.matmul(
                                dst,
                                kt[row0:row0 + 64,
                                   PT_OFF[tt]:PT_OFF[tt] + pk],
                                qt[qrow:qrow + 64, o:o + w],
                                start=True, stop=True,
                            )
                        if s < 2:
                            nc.scalar.activation(
                                E[0:pk, 0:1024], psA[0:pk, :], EXP)
                            nc.scalar.activation(
                                E[0:pk, 1024:1500], psB[0:pk, :], EXP)
                        else:
                            nc.scalar.activation(
                                E[0:pk, 0:768], psA[0:pk, 0:768], EXP)
                        for c, (o, w) in enumerate(sub):
                            nc.tensor.matmul(
                                wv_ps[c][0:66, 0:w],
                                V[0:pk, 256 * tt + 66 * s:
                                  256 * tt + 66 * s + 66],
                                E[0:pk, o:o + w],
                                start=(tt == 0), stop=(tt == 11),
                            )
                    for c, (o, w) in enumerate(sub):
                        qi = slice(o, o + w)
                        wsb = fin.tile([66, 512], f32, tag="wsb", name="wsb")
                        nc.vector.tensor_copy(wsb[0:66, 0:w],
                                              wv_ps[c][0:66, 0:w])
                        wps = wsb
                        # normalize: wv / Z, Z at partition 64; bounce the
                        # reciprocal row through DRAM to broadcast it across
                        # partitions (SBUF APs cannot have 0 partition step)
                        rz = fin.tile([65, 512], f32, tag="rz", name="rz")
                        nc.vector.reciprocal(rz[64:65, 0:w], wps[64:65, 0:w])
                        zscr = zdram.tile([1, 512], f32, tag="zscr",
                                          name="zscr")
                        nc.sync.dma_start(zscr[0:1, 0:w], rz[64:65, 0:w])
                        zs = zscr[0:1, 0:w]
                        zbc = bass.AP(
                            tensor=zs.tensor,
                            offset=zs.offset,
                            ap=[[0, 64]] + list(zs.ap),
                        )
                        rzb = fin.tile([64, 512], f32, tag="rzb", name="rzb")
                        nc.sync.dma_start(rzb[0:64, 0:w], zbc)
                        if s == 0:
                            nc.vector.tensor_mul(
                                WVT12[0:64, qi], wps[0:64, 0:w],
                                rzb[0:64, 0:w])
                        elif s == 1:
                            w2 = fin.tile([64, 512], mm_dt, tag="w2",
                                          name="w2")
                            nc.vector.tensor_mul(
                                w2[0:64, 0:w], wps[0:64, 0:w],
                                rzb[0:64, 0:w])
                            nc.sync.dma_start(WVT12[64:128, qi],
                                              w2[0:64, 0:w])
                        else:
                            nc.vector.tensor_mul(
                                WVT3[0:64, qi], wps[0:64, 0:w],
                                rzb[0:64, 0:w])

                # ---------------- out projections ----------------
                # emitted after slot 2, but the full-head pass depends only
                # on WVT12 (slots 0/1) and uses the banks freed by stB/wv2,
                # so the scheduler overlaps it with slot 2's attention
                with tc.tile_pool(name="ost", bufs=4) as ost, \
                     tc.tile_pool(name="pp_o", bufs=2, space="PSUM") as pp_o:
                    # full heads -> out; staging copies alternate DVE/ACT
                    ncopy = 0
                    for tt in range(12):
                        pk = PT[tt]
                        tsl = slice(PT_OFF[tt], PT_OFF[tt] + pk)
                        stage = ost.tile([128, F], f32, tag="stage",
                                         name="stage")
                        eoff = 0
                        for ec in ECH:
                            ps = pp_o.tile([128, ECH[0]], f32, tag="po",
                                           name="po")
                            nc.tensor.matmul(
                                ps[0:pk, 0:ec],
                                WVT12[:, tsl],
                                WO12[:, eoff:eoff + ec],
                                start=True, stop=True,
                            )
                            if ncopy % 3 != 2:
                                nc.vector.tensor_copy(
                                    stage[0:pk, eoff:eoff + ec],
                                    ps[0:pk, 0:ec])
                            else:
                                nc.scalar.copy(
                                    stage[0:pk, eoff:eoff + ec],
                                    ps[0:pk, 0:ec])
                            ncopy += 1
                            eoff += ec
                        nc.sync.dma_start(out_d[tsl, :], stage[0:pk, :])
                    # shared head -> out2 (rows 750:768 junk, dropped later)
                    for tt in range(6):
                        tsl = slice(128 * tt, 128 * (tt + 1))
                        stage = ost.tile([128, F], f32, tag="stage",
                                         name="stage")
                        eoff = 0
                        for ec in ECH:
                            ps = pp_o.tile([128, ECH[0]], f32, tag="po",
                                           name="po")
                            nc.tensor.matmul(
                                ps[:, 0:ec],
                                WVT3[:, tsl],
                                WO3[:, eoff:eoff + ec],
                                start=True, stop=True,
                            )
                            if ncopy % 3 != 2:
                                nc.vector.tensor_copy(
                                    stage[:, eoff:eoff + ec], ps[:, 0:ec])
                            else:
                                nc.scalar.copy(
                                    stage[:, eoff:eoff + ec], ps[:, 0:ec])
                            ncopy += 1
                            eoff += ec
                        nc.sync.dma_start(out2_d[tsl, :], stage[:, :])
                attn_ctx.close()

    nc.compile()
    return nc


def _get_nc(mm_dtype_name):
    if mm_dtype_name not in _CACHE:
        _CACHE[mm_dtype_name] = _build(mm_dtype_name)
    return _CACHE[mm_dtype_name]


def _prep_core_inputs(c, x, xT, WqT, bq, WkTs, WvT, bv, WoT):
    """Per-core inputs.  Full heads F0=2c, F1=2c+1; shared head S=16+c//2
    with q rows 750*(c%2) .. +750."""
    F0, F1 = 2 * c, 2 * c + 1
    S = 16 + c // 2
    roff = 750 * (c % 2)

    def hsl(h):
        return slice(D * h, D * (h + 1))

    wqk = np.zeros((F, 384), dtype=np.float32)
    bqk = np.zeros((384, 1), dtype=np.float32)
    wqk[:, 0:64] = WqT[:, hsl(F0)]
    wqk[:, 64:128] = WqT[:, hsl(F1)]
    bqk[0:64, 0] = bq[hsl(F0)]
    bqk[64:128, 0] = bq[hsl(F1)]
    wqk[:, 128:192] = WkTs[:, hsl(F0)]
    wqk[:, 192:256] = WkTs[:, hsl(F1)]
    wqk[:, 256:320] = WkTs[:, hsl(S)]

    wvw = np.zeros((F, 256), dtype=np.float32)
    bvr = np.zeros((256,), dtype=np.float32)
    for s, h in enumerate((F0, F1, S)):
        wvw[:, 66 * s:66 * s + 64] = WvT[:, hsl(h)]
        bvr[66 * s:66 * s + 64] = bv[hsl(h)]
        bvr[66 * s + 64] = 1.0

    qs = np.zeros((64, 768), dtype=np.float32)
    qs[:, 0:750] = (x[roff:roff + 750] @ WqT[:, hsl(S)] + bq[hsl(S)]).T

    wo = np.zeros((HS * D, F), dtype=np.float32)
    wo[0:64] = WoT[hsl(F0), :]
    wo[64:128] = WoT[hsl(F1), :]
    wo[128:192] = WoT[hsl(S), :]

    return {
        "xT": xT,
        "wqk": wqk,
        "bqk": bqk,
        "wvw": wvw,
        "bv": np.broadcast_to(bvr, (128, 256)).copy(),
        "qs": qs,
        "wo": wo,
    }


def _make_runner(nc):
    """Axon-path runner (built once, reused).  Three separate jits because
    neuronx_cc_hook requires the bass module to contain only the bass_exec
    custom call: (1) on-device zero output buffers, (2) the sharded bass
    call, (3) on-device combine: psum(out + place(out2)).  Only one [T, F]
    array is transferred back; per-core uploads are cached on device."""
    import jax
    import jax.numpy as jnp
    import concourse.mybir as mybir
    from concourse import bass2jax
    from jax.experimental.shard_map import shard_map
    from jax.sharding import Mesh, PartitionSpec

    bass2jax.install_neuronx_cc_hook()

    partition_name = (nc.partition_id_tensor.name
                      if nc.partition_id_tensor else None)

    REPLICATED = {"xT"}
    in_names, out_names, out_avals, zero_templates = [], [], [], []
    for alloc in nc.m.functions[0].allocations:
        if not isinstance(alloc, mybir.MemoryLocationSet):
            continue
        name = alloc.memorylocations[0].name
        if alloc.kind == "ExternalInput":
            if name != partition_name:
                in_names.append(name)
        elif alloc.kind == "ExternalOutput":
            out_names.append(name)
            shape = tuple(alloc.tensor_shape)
            dtype = mybir.dt.np(alloc.dtype)
            out_avals.append(jax.core.ShapedArray(shape, dtype))
            zero_templates.append((shape, dtype))
    n_params = len(in_names)
    n_outs = len(out_avals)
    all_names = in_names + out_names
    if partition_name is not None:
        all_names = all_names + [partition_name]
    donate = tuple(range(n_params, n_params + n_outs))
    i_out = out_names.index("out")
    i_out2 = out_names.index("out2")

    devices = jax.devices()[:NCORES]
    mesh = Mesh(np.asarray(devices), ("core",))

    def _body(*args):
        operands = list(args)
        if partition_name is not None:
            operands.append(bass2jax.partition_id_tensor())
        outs = bass2jax._bass_exec_p.bind(
            *operands,
            out_avals=tuple(out_avals),
            in_names=tuple(all_names),
            out_names=tuple(out_names),
            lowering_input_output_aliases=(),
            sim_require_finite=True,
            sim_require_nnan=True,
            nc=nc,
        )
        return tuple(outs)

    in_specs = tuple(
        PartitionSpec() if n in REPLICATED else PartitionSpec("core")
        for n in in_names
    ) + (PartitionSpec("core"),) * n_outs
    bass_fn = jax.jit(
        shard_map(_body, mesh=mesh, in_specs=in_specs,
                  out_specs=(PartitionSpec("core"),) * n_outs,
                  check_rep=False),
        donate_argnums=donate, keep_unused=True,
    )

    def _zeros():
        return tuple(jnp.zeros(s, d) for (s, d) in zero_templates)

    zeros_fn = jax.jit(
        shard_map(_zeros, mesh=mesh, in_specs=(),
                  out_specs=(PartitionSpec("core"),) * n_outs,
                  check_rep=False))

    def _combine(o, o2):
        idx = jax.lax.axis_index("core")
        off = 750 * (idx % 2)
        z = jnp.zeros((T, F), o.dtype)
        z = jax.lax.dynamic_update_slice(z, o2[0:750], (off, 0))
        return jax.lax.psum(o + z, "core")

    reduce_fn = jax.jit(
        shard_map(_combine, mesh=mesh,
                  in_specs=(PartitionSpec("core"), PartitionSpec("core")),
                  out_specs=PartitionSpec(), check_rep=False))

    dev_cache = {}

    def run(in_maps):
        args = []
        for n in in_names:
            if n in REPLICATED:
                arr = np.asarray(in_maps[0][n])
            else:
                arr = np.concatenate(
                    [np.asarray(in_maps[c][n]) for c in range(NCORES)],
                    axis=0)
            fp = (arr.shape, hash(arr.tobytes()))
            cached = dev_cache.get(n)
            if cached is not None and cached[0] == fp:
                args.append(cached[1])
            else:
                dev_arr = jax.device_put(
                    arr, jax.sharding.NamedSharding(
                        mesh,
                        PartitionSpec() if n in REPLICATED
                        else PartitionSpec("core")))
                dev_cache[n] = (fp, dev_arr)
                args.append(dev_arr)
        zeros = zeros_fn()
        outs = bass_fn(*args, *zeros)
        total = reduce_fn(outs[i_out], outs[i_out2])
        return np.asarray(total)

    return run


def kernel(x, Wq, bq, Wk, Wv, bv, Wo, bo):
    global LAST_RESULTS

    mm_dtype_name = os.environ.get("KERNEL_MM_DTYPE", "float32r")
    nc = _get_nc(mm_dtype_name)

    x = np.asarray(x, dtype=np.float32).reshape(T, F)
    xT = np.ascontiguousarray(x.T)
    WqT = np.ascontiguousarray(np.asarray(Wq, dtype=np.float32).T)
    WkTs = (np.ascontiguousarray(np.asarray(Wk, dtype=np.float32).T)
            * np.float32(QK_SCALE))
    WvT = np.ascontiguousarray(np.asarray(Wv, dtype=np.float32).T)
    WoT = np.ascontiguousarray(np.asarray(Wo, dtype=np.float32).T)
    bq = np.asarray(bq, dtype=np.float32)
    bvv = np.asarray(bv, dtype=np.float32)

    in_maps = [
        _prep_core_inputs(c, x, xT, WqT, bq, WkTs, WvT, bvv, WoT)
        for c in range(NCORES)
    ]

    from concourse._compat import axon_active

    if axon_active():
        key = (mm_dtype_name, "runner")
        if key not in _CACHE:
            _CACHE[key] = _make_runner(nc)
        out = np.array(_CACHE[key](in_maps), dtype=np.float32)
    else:
        from concourse.bass_utils import run_bass_kernel_spmd
        trace = os.environ.get("KERNEL_TRACE", "0") == "1"
        res = run_bass_kernel_spmd(nc, in_maps, core_ids=list(range(NCORES)),
                                   trace=trace)
        LAST_RESULTS = res
        out = np.zeros((T, F), dtype=np.float32)
        for c in range(NCORES):
            out += res.results[c]["out"]
            roff = 750 * (c % 2)
            out[roff:roff + 750] += res.results[c]["out2"][0:750]
    out += np.asarray(bo, dtype=np.float32)
    return out.reshape(1, T, F)



# revision 27
# speedup vs baseline: 1.0530x; 1.0530x over previous
"""Multi-head attention (B=1, T=1500, N=1280, H=20, D=64) on 8 NeuronCores.

Tensor-parallel by head groups, 2.5 heads/core: core c owns full heads
F0=2c, F1=2c+1 plus half of shared head S=16+c//2 (q rows 750*(c%2)..+750).

v2 design (all matmuls bf16; cost model: PE charges N output columns per
matmul at 1 cycle/col regardless of K and M):
  - q/k projections computed transposed (3 m-chunks of 128 rows x 1500);
    the shared head's half-q is tiny and computed on the host (per-core
    q window), passed in as `qs`,
  - v projection in natural [t, d] layout, 3 slots x 66 cols (64 v + ones
    column for the softmax denominator + pad),
  - attention is software-pipelined by head: window h runs scores+exp of
    head h on PE+ACT while PE sweeps head h-1's wv in the flipped [q, d]
    layout (qt-outer / kt-inner accumulation, one PSUM bank), normalizes
    per-partition (reciprocal + tensor_scalar_mul; Z is wv column 64),
    PE-transposes [q,64] -> [64,q] into WVT, and runs fillers (v-proj,
    out-proj) in the ACT-paced slack,
  - exp without max subtraction (scores ~N(0,1), max < ~7),
  - out-projection: full heads K=128 over 12 t-tiles, shared head K=64
    over 6 q-tiles of 125; PSUM->SBUF copies rotate DVE/ACT/Pool; DMA out.
"""

import os

import numpy as np

T, F, D = 1500, 1280, 64
NH = 20
QK_SCALE = D ** (-0.5)
NCORES = 8

PT = [128] * 11 + [92]          # partition tiles along t (sum = 1500)
PT_OFF = [128 * i for i in range(12)]
FULLSUB = [(0, 512), (512, 512), (1024, 476)]
SSUB = [(0, 512), (512, 238)]   # shared head q window = 750
SQT = 125                       # shared head q-tile width (6 x 125 = 750)
OCH = [(0, 512), (512, 512), (1024, 256)]
VSLOT = 66                      # 64 v cols + ones + pad
VW = 3 * VSLOT                  # 198

DEFAULT_MM_DTYPE = "bfloat16"

_CACHE = {}
LAST_RESULTS = None


def _build(mm_dtype_name):
    import concourse.bacc as bacc
    import concourse.mybir as mybir
    import concourse.tile as tile

    f32 = mybir.dt.float32
    mm_dt = getattr(mybir.dt, mm_dtype_name)

    nc = bacc.Bacc("TRN2", target_bir_lowering=False, debug=False,
                   num_devices=NCORES)

    xT_d = nc.dram_tensor("xT", [F, T], mm_dt, kind="ExternalInput").ap()
    wqk_d = nc.dram_tensor("wqk", [F, 384], mm_dt, kind="ExternalInput").ap()
    bqk_d = nc.dram_tensor("bqk", [128, 1], f32, kind="ExternalInput").ap()
    qs_d = nc.dram_tensor("qs", [64, 750], mm_dt, kind="ExternalInput").ap()
    wvw_d = nc.dram_tensor("wvw", [F, VW], mm_dt, kind="ExternalInput").ap()
    bv_d = nc.dram_tensor("bv", [1, VW], f32, kind="ExternalInput").ap()
    wo_d = nc.dram_tensor("wo", [192, F], mm_dt, kind="ExternalInput").ap()
    idn_d = nc.dram_tensor("idn", [128, 128], mm_dt,
                           kind="ExternalInput").ap()
    out_d = nc.dram_tensor("out", [T, F], f32, kind="ExternalOutput").ap()
    out2_d = nc.dram_tensor("out2", [750, F], f32, kind="ExternalOutput").ap()

    EXP = mybir.ActivationFunctionType.Exp

    import concourse.bass as bass  # noqa: F401

    with tile.TileContext(nc) as tc:
        from contextlib import ExitStack
        with ExitStack() as ctx:
            persist = ctx.enter_context(tc.tile_pool(name="persist", bufs=1))

            # ---------------- persistent SBUF ----------------
            XT = [persist.tile([128, T], mm_dt, tag=f"xt{i}", name=f"xt{i}")
                  for i in range(10)]
            WQK = [persist.tile([128, 384], mm_dt, tag=f"wqk{i}",
                                name=f"wqk{i}") for i in range(10)]
            WVW = [persist.tile([128, VW], mm_dt, tag=f"wvw{i}",
                                name=f"wvw{i}") for i in range(10)]
            Q12 = persist.tile([128, T], mm_dt, tag="q12", name="q12")
            K12 = persist.tile([128, T], mm_dt, tag="k12", name="k12")
            K3 = persist.tile([64, T], mm_dt, tag="k3", name="k3")
            QS = persist.tile([64, 750], mm_dt, tag="qs", name="qs")
            V = persist.tile([128, 12 * VW], mm_dt, tag="vall", name="vall")
            WVT12 = persist.tile([128, T], mm_dt, tag="wvt12", name="wvt12")
            WVT3 = persist.tile([64, 750], mm_dt, tag="wvt3", name="wvt3")
            WO12 = persist.tile([128, F], mm_dt, tag="wo12", name="wo12")
            WO3 = persist.tile([64, F], mm_dt, tag="wo3", name="wo3")
            BQK = persist.tile([128, 1], f32, tag="bqk", name="bqk")
            BV = persist.tile([128, VW], f32, tag="bv", name="bv")
            IDN = persist.tile([128, 128], mm_dt, tag="idn", name="idn")

            epool = ctx.enter_context(tc.tile_pool(name="epool", bufs=24))
            fin = ctx.enter_context(tc.tile_pool(name="fin", bufs=4))
            ost = ctx.enter_context(tc.tile_pool(name="ost", bufs=2))

            # ---------------- input DMAs ----------------
            for i in range(10):
                nc.sync.dma_start(WQK[i][:], wqk_d[128 * i:128 * (i + 1), :])
            for i in range(10):
                nc.sync.dma_start(XT[i][:], xT_d[128 * i:128 * (i + 1), :])
            nc.sync.dma_start(BQK[:], bqk_d[:, :])
            nc.sync.dma_start(QS[:], qs_d[:, :])
            nc.sync.dma_start(IDN[:], idn_d[:, :])
            for i in range(10):
                nc.sync.dma_start(WVW[i][:], wvw_d[128 * i:128 * (i + 1), :])
            bvs = bv_d[0:1, :]
            bv_bc = bass.AP(tensor=bvs.tensor, offset=bvs.offset,
                            ap=[[0, 128]] + list(bvs.ap)[1:])
            nc.sync.dma_start(BV[:], bv_bc)
            nc.sync.dma_start(WO12[:], wo_d[0:128, :])
            nc.sync.dma_start(WO3[:], wo_d[128:192, :])

            # ---------------- q/k projections (lead-in) ----------------
            # m0 = [q_F0|q_F1], m1 = [k_F0|k_F1], m2 = [k_S|q_S]
            qk_ctx = ExitStack()
            pp_qk = qk_ctx.enter_context(
                tc.tile_pool(name="pp_qk", bufs=3, space="PSUM"))
            for m in (0, 1, 2):
                pss = [pp_qk.tile([128, 512], f32, tag="pqk", name="pqk")
                       for _ in range(3)]
                for f in range(10):
                    for ci, (o, w) in enumerate(FULLSUB):
                        nc.tensor.matmul(
                            pss[ci][0:128, 0:w],
                            WQK[f][:, 128 * m:128 * (m + 1)],
                            XT[f][:, o:o + w],
                            start=(f == 0), stop=(f == 9),
                        )
                for ci, (o, w) in enumerate(FULLSUB):
                    if m == 0:
                        nc.vector.tensor_scalar_add(
                            Q12[:, o:o + w], pss[ci][0:128, 0:w],
                            BQK[:, 0:1])
                    elif m == 1:
                        # k has no bias
                        nc.scalar.copy(K12[:, o:o + w], pss[ci][0:128, 0:w])
                    else:
                        nc.scalar.copy(K3[0:64, o:o + w],
                                       pss[ci][0:64, 0:w])
            qk_ctx.close()

            # ---------------- pipelined attention ----------------
            # window h: scores+exp(h) | wv+norm+transpose(h-1) | fillers
            att_ctx = ExitStack()
            sc_ctx = ExitStack()
            sc_pool = [sc_ctx.enter_context(
                tc.tile_pool(name="pp_sc", bufs=2, space="PSUM"))]
            vp_ctx = ExitStack()
            pp_v = vp_ctx.enter_context(
                tc.tile_pool(name="pp_v", bufs=2, space="PSUM"))
            pp_att = None  # opened after the v-proj pool closes (PSUM banks)

            E12 = [[None] * 12 for _ in range(3)]
            # PSUM is only reachable from DVE/ACT/PE; rotate psum->sbuf
            # copies 2:1 DVE:ACT (ACT is busy with exp)
            cp_rr = [0]

            def copy_eng():
                cp_rr[0] += 1
                if cp_rr[0] % 3 == 0:
                    return nc.scalar.copy
                return nc.vector.tensor_copy

            def emit_scores(h, kt):
                pk = PT[kt]
                wq = T if h < 2 else 750
                ps = sc_pool[0].tile([128, wq], f32, tag="psc", name="psc")
                sub = FULLSUB if h < 2 else SSUB
                if h < 2:
                    lh = K12[64 * h:64 * h + 64, PT_OFF[kt]:PT_OFF[kt] + pk]
                    qsrc = Q12[64 * h:64 * h + 64, :]
                else:
                    lh = K3[0:64, PT_OFF[kt]:PT_OFF[kt] + pk]
                    qsrc = QS[0:64, :]
                for (o, w) in sub:
                    nc.tensor.matmul(
                        ps[0:pk, o:o + w],
                        lh,
                        qsrc[:, o:o + w],
                        start=True, stop=True,
                    )
                E = epool.tile([128, T], mm_dt, tag="E", name="E")
                nc.scalar.activation(E[0:pk, 0:wq], ps[0:pk, 0:wq], EXP)
                E12[h][kt] = E

            def emit_vproj(tt):
                pk = PT[tt]
                ps = pp_v.tile([128, VW], f32, tag="pv", name="pv")
                for f in range(10):
                    nc.tensor.matmul(
                        ps[0:pk, :],
                        XT[f][:, PT_OFF[tt]:PT_OFF[tt] + pk],
                        WVW[f][:],
                        start=(f == 0), stop=(f == 9),
                    )
                nc.vector.tensor_add(
                    V[0:pk, VW * tt:VW * (tt + 1)], ps[0:pk, :], BV[0:pk, :])

            def emit_wv_norm(h, qt):
                """wv for head h, q-tile qt (qt-outer, kt-inner accumulate),
                then normalize + transpose into WVT."""
                if h < 2:
                    pkq = PT[qt]
                    qo = PT_OFF[qt]
                else:
                    pkq = SQT
                    qo = SQT * qt
                ps = pp_att.tile([128, VSLOT], f32, tag="wv", name="wv")
                for kt in range(12):
                    pkk = PT[kt]
                    nc.tensor.matmul(
                        ps[0:pkq, 0:VSLOT],
                        E12[h][kt][0:pkk, qo:qo + pkq],
                        V[0:pkk, VW * kt + VSLOT * h:
                          VW * kt + VSLOT * h + VSLOT],
                        start=(kt == 0), stop=(kt == 11),
                    )
                rz = fin.tile([128, 1], f32, tag="rz", name="rz")
                nc.vector.reciprocal(rz[0:pkq, 0:1], ps[0:pkq, 64:65])
                wsb = fin.tile([128, 64], mm_dt, tag="wsb", name="wsb")
                nc.vector.tensor_scalar_mul(wsb[0:pkq, 0:64],
                                            ps[0:pkq, 0:64], rz[0:pkq, 0:1])
                pst = pp_att.tile([64, 128], mm_dt, tag="pt", name="pt")
                nc.tensor.transpose(pst[0:64, 0:pkq], wsb[0:pkq, 0:64],
                                    IDN[0:pkq, 0:pkq])
                cp = nc.scalar.copy if qt % 2 == 0 else nc.vector.tensor_copy
                if h < 2:
                    cp(WVT12[64 * h:64 * h + 64, qo:qo + pkq],
                       pst[0:64, 0:pkq])
                else:
                    cp(WVT3[0:64, qo:qo + pkq], pst[0:64, 0:pkq])

            def emit_outproj12(tt):
                pk = PT[tt]
                tsl = slice(PT_OFF[tt], PT_OFF[tt] + pk)
                stage = ost.tile([128, F], f32, tag="stage", name="stage")
                for (o, w) in OCH:
                    ps = pp_o.tile([128, 512], f32, tag="po", name="po")
                    nc.tensor.matmul(
                        ps[0:pk, 0:w], WVT12[:, tsl], WO12[:, o:o + w],
                        start=True, stop=True,
                    )
                    copy_eng()(stage[0:pk, o:o + w], ps[0:pk, 0:w])
                nc.sync.dma_start(out_d[tsl, :], stage[0:pk, :])

            def emit_outproj3(qt):
                qsl = slice(SQT * qt, SQT * (qt + 1))
                stage = ost.tile([128, F], f32, tag="stage", name="stage")
                for (o, w) in OCH:
                    ps = pp_o.tile([128, 512], f32, tag="po", name="po")
                    nc.tensor.matmul(
                        ps[0:SQT, 0:w], WVT3[:, qsl], WO3[:, o:o + w],
                        start=True, stop=True,
                    )
                    copy_eng()(stage[0:SQT, o:o + w], ps[0:SQT, 0:w])
                nc.sync.dma_start(out2_d[qsl, :], stage[0:SQT, :])

            # window 0: head F0 scores/exp + v-proj fillers
            for kt in range(12):
                emit_scores(0, kt)
                emit_vproj(kt)
            vp_ctx.close()
            pp_att = att_ctx.enter_context(
                tc.tile_pool(name="pp_att", bufs=1, space="PSUM",
                             side="right"))

            # window 1: head F1 scores/exp + F0 wv/norm/transpose
            for kt in range(12):
                emit_scores(1, kt)
                emit_wv_norm(0, kt)

            # window 2: head S scores/exp + F1 wv + out-proj of full heads
            # (swap the big scores pool for a smaller one + out-proj psum)
            sc_ctx.close()
            sc_ctx = ExitStack()
            sc_pool[0] = sc_ctx.enter_context(
                tc.tile_pool(name="pp_scs", bufs=2, space="PSUM"))
            o_ctx = ExitStack()
            pp_o = o_ctx.enter_context(
                tc.tile_pool(name="pp_o", bufs=2, space="PSUM"))
            for kt in range(12):
                emit_scores(2, kt)
                emit_wv_norm(1, kt)
                if kt >= 1:
                    emit_outproj12(kt - 1)

            # drain: S wv + remaining out-projections
            for qt in range(6):
                emit_wv_norm(2, qt)
                if qt == 0:
                    emit_outproj12(11)
                else:
                    emit_outproj3(qt - 1)
            emit_outproj3(5)
            o_ctx.close()
            sc_ctx.close()
            att_ctx.close()

    nc.compile()
    return nc


def _get_nc(mm_dtype_name=None):
    if mm_dtype_name is None:
        mm_dtype_name = DEFAULT_MM_DTYPE
    if mm_dtype_name not in _CACHE:
        _CACHE[mm_dtype_name] = _build(mm_dtype_name)
    return _CACHE[mm_dtype_name]


def _to_mm(a, mm_dtype_name):
    import ml_dtypes
    dt = {"bfloat16": ml_dtypes.bfloat16, "float32r": np.float32,
          "float32": np.float32}[mm_dtype_name]
    return np.ascontiguousarray(np.asarray(a, dtype=np.float32)).astype(dt)


def _prep_core_inputs(c, x, xT, WqT, bq, WkTs, WvT, bv, WoT, mm_dtype_name):
    """Per-core inputs.  Full heads F0=2c, F1=2c+1; shared head S=16+c//2
    with q rows 750*(c%2) .. +750 (qoff only affects the kernel's q window,
    encoded at build time)."""
    F0, F1 = 2 * c, 2 * c + 1
    S = 16 + c // 2

    def hsl(h):
        return slice(D * h, D * (h + 1))

    wqk = np.zeros((F, 384), dtype=np.float32)
    wqk[:, 0:64] = WqT[:, hsl(F0)]
    wqk[:, 64:128] = WqT[:, hsl(F1)]
    wqk[:, 128:192] = WkTs[:, hsl(F0)]
    wqk[:, 192:256] = WkTs[:, hsl(F1)]
    wqk[:, 256:320] = WkTs[:, hsl(S)]

    bqk = np.zeros((128, 1), dtype=np.float32)
    bqk[0:64, 0] = bq[hsl(F0)]
    bqk[64:128, 0] = bq[hsl(F1)]

    roff = 750 * (c % 2)
    qs = (x[roff:roff + 750] @ WqT[:, hsl(S)] + bq[hsl(S)]).T

    wvw = np.zeros((F, VW), dtype=np.float32)
    bvr = np.zeros((1, VW), dtype=np.float32)
    for s, h in enumerate((F0, F1, S)):
        wvw[:, VSLOT * s:VSLOT * s + 64] = WvT[:, hsl(h)]
        bvr[0, VSLOT * s:VSLOT * s + 64] = bv[hsl(h)]
        bvr[0, VSLOT * s + 64] = 1.0

    wo = np.zeros((192, F), dtype=np.float32)
    wo[0:64] = WoT[hsl(F0), :]
    wo[64:128] = WoT[hsl(F1), :]
    wo[128:192] = WoT[hsl(S), :]

    idn = np.eye(128, dtype=np.float32)

    return {
        "xT": xT,
        "wqk": _to_mm(wqk, mm_dtype_name),
        "bqk": bqk,
        "qs": _to_mm(qs, mm_dtype_name),
        "wvw": _to_mm(wvw, mm_dtype_name),
        "bv": bvr,
        "wo": _to_mm(wo, mm_dtype_name),
        "idn": _to_mm(idn, mm_dtype_name),
    }


def _make_runner(nc):
    """Axon-path runner (built once, reused).  Three separate jits because
    neuronx_cc_hook requires the bass module to contain only the bass_exec
    custom call: (1) on-device zero output buffers, (2) the sharded bass
    call, (3) on-device combine: psum(out + place(out2)).  Only one [T, F]
    array is transferred back; per-core uploads are cached on device."""
    import jax
    import jax.numpy as jnp
    import concourse.mybir as mybir
    from concourse import bass2jax
    from jax.experimental.shard_map import shard_map
    from jax.sharding import Mesh, PartitionSpec

    bass2jax.install_neuronx_cc_hook()

    partition_name = (nc.partition_id_tensor.name
                      if nc.partition_id_tensor else None)

    REPLICATED = {"xT", "idn"}
    in_names, out_names, out_avals, zero_templates = [], [], [], []
    for alloc in nc.m.functions[0].allocations:
        if not isinstance(alloc, mybir.MemoryLocationSet):
            continue
        name = alloc.memorylocations[0].name
        if alloc.kind == "ExternalInput":
            if name != partition_name:
                in_names.append(name)
        elif alloc.kind == "ExternalOutput":
            out_names.append(name)
            shape = tuple(alloc.tensor_shape)
            dtype = mybir.dt.np(alloc.dtype)
            out_avals.append(jax.core.ShapedArray(shape, dtype))
            zero_templates.append((shape, dtype))
    n_params = len(in_names)
    n_outs = len(out_avals)
    all_names = in_names + out_names
    if partition_name is not None:
        all_names = all_names + [partition_name]
    donate = tuple(range(n_params, n_params + n_outs))
    i_out = out_names.index("out")
    i_out2 = out_names.index("out2")

    devices = jax.devices()[:NCORES]
    mesh = Mesh(np.asarray(devices), ("core",))

    def _body(*args):
        operands = list(args)
        if partition_name is not None:
            operands.append(bass2jax.partition_id_tensor())
        outs = bass2jax._bass_exec_p.bind(
            *operands,
            out_avals=tuple(out_avals),
            in_names=tuple(all_names),
            out_names=tuple(out_names),
            lowering_input_output_aliases=(),
            sim_require_finite=True,
            sim_require_nnan=True,
            nc=nc,
        )
        return tuple(outs)

    in_specs = tuple(
        PartitionSpec() if n in REPLICATED else PartitionSpec("core")
        for n in in_names
    ) + (PartitionSpec("core"),) * n_outs
    bass_fn = jax.jit(
        shard_map(_body, mesh=mesh, in_specs=in_specs,
                  out_specs=(PartitionSpec("core"),) * n_outs,
                  check_rep=False),
        donate_argnums=donate, keep_unused=True,
    )

    def _zeros():
        return tuple(jnp.zeros(s, d) for (s, d) in zero_templates)

    zeros_fn = jax.jit(
        shard_map(_zeros, mesh=mesh, in_specs=(),
                  out_specs=(PartitionSpec("core"),) * n_outs,
                  check_rep=False))

    def _combine(o, o2):
        idx = jax.lax.axis_index("core")
        off = 750 * (idx % 2)
        z = jnp.zeros((T, F), o.dtype)
        z = jax.lax.dynamic_update_slice(z, o2[0:750], (off, 0))
        return jax.lax.psum(o + z, "core")

    reduce_fn = jax.jit(
        shard_map(_combine, mesh=mesh,
                  in_specs=(PartitionSpec("core"), PartitionSpec("core")),
                  out_specs=PartitionSpec(), check_rep=False))

    dev_cache = {}

    def run(in_maps):
        args = []
        for n in in_names:
            if n in REPLICATED:
                arr = np.asarray(in_maps[0][n])
            else:
                arr = np.concatenate(
                    [np.asarray(in_maps[c][n]) for c in range(NCORES)],
                    axis=0)
            fp = (arr.shape, hash(arr.tobytes()))
            cached = dev_cache.get(n)
            if cached is not None and cached[0] == fp:
                args.append(cached[1])
            else:
                dev_arr = jax.device_put(
                    arr, jax.sharding.NamedSharding(
                        mesh,
                        PartitionSpec() if n in REPLICATED
                        else PartitionSpec("core")))
                dev_cache[n] = (fp, dev_arr)
                args.append(dev_arr)
        zeros = zeros_fn()
        outs = bass_fn(*args, *zeros)
        total = reduce_fn(outs[i_out], outs[i_out2])
        return np.asarray(total)

    return run


def kernel(x, Wq, bq, Wk, Wv, bv, Wo, bo):
    global LAST_RESULTS

    mm_dtype_name = os.environ.get("KERNEL_MM_DTYPE", DEFAULT_MM_DTYPE)
    nc = _get_nc(mm_dtype_name)

    x = np.asarray(x, dtype=np.float32).reshape(T, F)
    xT = _to_mm(np.ascontiguousarray(x.T), mm_dtype_name)
    WqT = np.ascontiguousarray(np.asarray(Wq, dtype=np.float32).T)
    WkTs = (np.ascontiguousarray(np.asarray(Wk, dtype=np.float32).T)
            * np.float32(QK_SCALE))
    WvT = np.ascontiguousarray(np.asarray(Wv, dtype=np.float32).T)
    WoT = np.ascontiguousarray(np.asarray(Wo, dtype=np.float32).T)
    bq = np.asarray(bq, dtype=np.float32)
    bvv = np.asarray(bv, dtype=np.float32)

    in_maps = [
        _prep_core_inputs(c, x, xT, WqT, bq, WkTs, WvT, bvv, WoT,
                          mm_dtype_name)
        for c in range(NCORES)
    ]

    from concourse._compat import axon_active

    if axon_active():
        key = (mm_dtype_name, "runner")
        if key not in _CACHE:
            _CACHE[key] = _make_runner(nc)
        out = np.array(_CACHE[key](in_maps), dtype=np.float32)
    else:
        from concourse.bass_utils import run_bass_kernel_spmd
        trace = os.environ.get("KERNEL_TRACE", "0") == "1"
        res = run_bass_kernel_spmd(nc, in_maps, core_ids=list(range(NCORES)),
                                   trace=trace)
        LAST_RESULTS = res
        out = np.zeros((T, F), dtype=np.float32)
        for c in range(NCORES):
            out += res.results[c]["out"]
            roff = 750 * (c % 2)
            out[roff:roff + 750] += res.results[c]["out2"]
    out += np.asarray(bo, dtype=np.float32)
    return out.reshape(1, T, F)
